# revision 1
# baseline (speedup 1.0000x reference)
"""GeneSAGE (2-layer GraphSAGE + skip + LayerNorm + ELU) on 8 Trainium2 cores.

Strategy: edge-parallel by *destination range*. Core c owns nodes
[CP*c, CP*(c+1)) with CP=6272 (=49*128). Edges are bucketed host-side by
(dst-core, src-half, dst-window) and padded to 128-edge chunks with a chunk
structure common to all 8 cores (SPMD: one program). On device, per chunk:
one-hot(dst) built on DVE, segment-sum done as one-hot matmuls accumulating
in PSUM per 128-node window. Features are gathered from HBM with
dma_gather (256B rows, int16 indices over two half tables). Conv2 gathers
from an on-device-built table pb2 whose rows replicate p=h@W2l 32x so rows
are 256B. The only collective is an AllGather of p^T (2 x 6272 per core).
"""

import numpy as np

import concourse.mybir as mybir
from concourse import bacc, bass, tile
from concourse.bass_utils import run_bass_kernel_spmd

F32 = mybir.dt.float32
I16 = mybir.dt.int16

N_CORES = 8
D = 64          # input feature dim
HID = 256
OUT = 2
LN_EPS = 1e-5
BATCH_CHUNKS = 32   # chunks per dma_gather call
STAGE_CHUNKS = 0    # set per-plan: pb2 staging chunks per DMA


def make_plan(edge_index: np.ndarray, n_nodes: int):
    """Host-side edge bucketing. Returns plan dict with per-core arrays and
    the (common) chunk schedule."""
    # padded per-core node count: multiple of 128, 8 cores cover all nodes
    cp = int(np.ceil(n_nodes / (N_CORES * 128))) * 128
    nw = cp // 128
    npad = N_CORES * cp
    half = npad // 2
    assert half <= 32768, "int16 gather index limit"

    src = edge_index[0].astype(np.int64)
    dst = edge_index[1].astype(np.int64)
    E = src.shape[0]

    core = dst // cp
    stream = (src >= half).astype(np.int64)
    win = (dst % cp) // 128
    ngrp_per_core = 2 * nw
    key = (core * 2 + stream) * nw + win
    order = np.argsort(key, kind="stable")
    counts = np.bincount(key, minlength=N_CORES * ngrp_per_core).reshape(
        N_CORES, 2, nw
    )
    # common chunk count per (stream, window): max over cores
    nchunks = -(-counts.max(axis=0) // 128)  # [2, nw] ceil-div
    # chunk offsets, stream-major
    off = np.zeros((2, nw), np.int64)
    running = 0
    for s in range(2):
        for w in range(nw):
            off[s, w] = running
            running += nchunks[s, w]
    c_total = int(running)
    c_lo = int(nchunks[0].sum())
    e_slots = c_total * 128

    # per-edge slot position
    sk = key[order]
    grp_start = np.searchsorted(sk, np.arange(N_CORES * ngrp_per_core))
    rank = np.arange(E) - grp_start[sk]
    s_of = (sk // nw) % 2
    w_of = sk % nw
    c_of = sk // ngrp_per_core
    slot = off[s_of, w_of] * 128 + rank

    gidx = np.zeros((N_CORES, e_slots), np.int16)
    dstf = np.full((N_CORES, e_slots), -1.0, np.float32)
    gidx[c_of, slot] = (src[order] - s_of * half).astype(np.int16)
    dstf[c_of, slot] = (dst[order] % cp - w_of * 128).astype(np.float32)

    # device layouts
    # gather idx tile [128, e_slots//16]: tile[p, j] = gidx[16*j + p%16]
    a = gidx.reshape(N_CORES, e_slots // 16, 16).transpose(0, 2, 1)  # [c,16,J]
    gidx_tile = np.tile(a, (1, 8, 1)).copy()  # [c, 128, J]
    dstf_tile = (
        dstf.reshape(N_CORES, c_total, 128).transpose(0, 2, 1).copy()
    )  # [c, 128, C]

    # per (stream, window) first/last chunk ids (global chunk index), or None
    sched = []  # list over streams of list of (window, first_chunk, last_chunk)
    for s in range(2):
        rows = []
        for w in range(nw):
            n = int(nchunks[s, w])
            if n == 0:
                continue
            first = int(off[s, w])
            rows.append((w, first, first + n - 1))
        sched.append(rows)

    chunk_window = np.zeros(c_total, np.int64)
    for s in range(2):
        for w, f, l in sched[s]:
            chunk_window[f : l + 1] = w

    return dict(
        cp=cp, nw=nw, npad=npad, half=half,
        c_total=c_total, c_lo=c_lo,
        sched=sched, chunk_window=chunk_window,
        gidx_tile=gidx_tile, dstf_tile=dstf_tile,
        nchunks=nchunks,
    )


def build_program(plan):
    cp, nw, half = plan["cp"], plan["nw"], plan["half"]
    c_total, c_lo = plan["c_total"], plan["c_lo"]
    sched, chunk_window = plan["sched"], plan["chunk_window"]
    J = c_total * 8

    nc = bacc.Bacc("TRN2", target_bir_lowering=False, debug=False,
                   num_devices=N_CORES)

    def inp(name, shape, dt=F32):
        return nc.dram_tensor(name, shape, dt, kind="ExternalInput").ap()

    x_lo = inp("x_lo", [half, D])
    x_hi = inp("x_hi", [half, D])
    x_loc = inp("x_loc", [cp, D])
    gidx_d = inp("gidx", [128, J], I16)
    dstf_d = inp("dstf", [128, c_total])
    iota_d = inp("iota", [128, 128])
    ident_d = inp("ident", [128, 128])
    wcb_d = inp("wcb", [D + 1, HID])       # [Wc; bc]
    w1l_d = inp("w1l", [D, HID])
    w2lr_d = inp("w2lr", [128, 2 * 2 * OUT])  # halves of [W2l|W2r] packed
    gamma_d = inp("gamma_bc", [128, HID])
    beta_d = inp("beta_bc", [128, HID])
    b2_d = inp("b2_bc", [128, OUT])
    i2_d = inp("i2", [2, D])               # interleave pattern
    out_d = nc.dram_tensor("out", [cp, OUT], F32, kind="ExternalOutput").ap()

    with tile.TileContext(nc) as tc:
        with (
            tc.tile_pool(name="res", bufs=1) as res,
            tc.tile_pool(name="dram", bufs=1, space="DRAM") as dram,
        ):
            # ---- resident tiles / constants
            gidx_sb = res.tile([128, J], I16)
            nc.sync.dma_start(out=gidx_sb[:], in_=gidx_d[:])
            dstf_sb = res.tile([128, c_total], F32)
            nc.sync.dma_start(out=dstf_sb[:], in_=dstf_d[:])
            iota_sb = res.tile([128, 128], F32)
            nc.sync.dma_start(out=iota_sb[:], in_=iota_d[:])
            ident_sb = res.tile([128, 128], F32)
            nc.sync.dma_start(out=ident_sb[:], in_=ident_d[:])
            wcb_sb = res.tile([D + 1, HID], F32)
            nc.sync.dma_start(out=wcb_sb[:], in_=wcb_d[:])
            w1l_sb = res.tile([D, HID], F32)
            nc.sync.dma_start(out=w1l_sb[:], in_=w1l_d[:])
            w2lr_sb = res.tile([128, 2 * 2 * OUT], F32)
            nc.sync.dma_start(out=w2lr_sb[:], in_=w2lr_d[:])
            gamma_sb = res.tile([128, HID], F32)
            nc.sync.dma_start(out=gamma_sb[:], in_=gamma_d[:])
            beta_sb = res.tile([128, HID], F32)
            nc.sync.dma_start(out=beta_sb[:], in_=beta_d[:])
            b2_sb = res.tile([128, OUT], F32)
            nc.sync.dma_start(out=b2_sb[:], in_=b2_d[:])
            i2_sb = res.tile([2, D], F32)
            nc.sync.dma_start(out=i2_sb[:], in_=i2_d[:])
            xloc_sb = res.tile([128, nw, D], F32)
            nc.sync.dma_start(
                out=xloc_sb[:],
                in_=x_loc.rearrange("(w p) d -> p w d", p=128),
            )
            ones_sb = res.tile([128, 1], F32)
            nc.vector.memset(ones_sb[:], 1.0)

            aggcnt = res.tile([128, nw, D + 1], F32)
            nc.vector.memset(aggcnt[:], 0.0)
            agg2 = res.tile([128, nw, OUT], F32)
            nc.vector.memset(agg2[:], 0.0)
            rc_sb = res.tile([128, nw, 1], F32)
            h_sb = res.tile([128, nw, HID], F32)
            pr_sb = res.tile([128, nw, 2 * OUT], F32)
            out_sb = res.tile([128, nw, OUT], F32)

            pt_dram = dram.tile([2, cp], F32)
            ptall_dram = dram.tile([2 * N_CORES, cp], F32)
            pb2_lo = dram.tile([half, D], F32)
            pb2_hi = dram.tile([half, D], F32)

            # ---- generic aggregation pass over the edge stream
            def aggregation(tables, acc_tile, width, with_cnt):
                """tables: (lo_ap, hi_ap); acc_tile[:, w, :] accumulated into.
                width: feature width gathered (cols 0:width of 64 used).
                with_cnt: also matmul ones into acc col `width`."""
                with (
                    tc.tile_pool(name="gpool", bufs=3) as gpool,
                    tc.tile_pool(name="opool", bufs=4) as opool,
                    tc.tile_pool(name="pwpool", bufs=2, space="PSUM") as pwp,
                ):
                    for s in range(2):
                        table = tables[s]
                        rows = sched[s]
                        if not rows:
                            continue
                        c0 = rows[0][1]
                        c1 = rows[-1][2] + 1
                        # gather batches
                        gbufs = {}
                        for b0 in range(c0, c1, BATCH_CHUNKS):
                            b1 = min(b0 + BATCH_CHUNKS, c1)
                            g = gpool.tile([128, BATCH_CHUNKS, D], F32,
                                           tag="gbuf")
                            n_idx = (b1 - b0) * 128
                            nc.gpsimd.dma_gather(
                                out_ap=g[:, 0 : b1 - b0, :],
                                in_ap=table,
                                idxs_ap=gidx_sb[:, b0 * 8 : b1 * 8],
                                num_idxs=n_idx,
                                num_idxs_reg=n_idx,
                                elem_size=D,
                                single_packet=False,
                            )
                            gbufs[b0] = g
                        # per-window accumulation
                        for w, first, last in rows:
                            pw = pwp.tile([128, D], F32, tag="pw")
                            pwc = None
                            if with_cnt:
                                pwc = pwp.tile([128, 1], F32, tag="pwc",
                                               name="pwc")
                            for g in range(first, last + 1):
                                b0 = c0 + ((g - c0) // BATCH_CHUNKS) * BATCH_CHUNKS
                                gb = gbufs[b0]
                                o = opool.tile([128, 128], F32, tag="O")
                                nc.vector.tensor_scalar(
                                    out=o[:],
                                    in0=iota_sb[:],
                                    scalar1=dstf_sb[:, g : g + 1],
                                    scalar2=None,
                                    op0=mybir.AluOpType.is_equal,
                                )
                                st = g == first
                                sp = g == last
                                nc.tensor.matmul(
                                    pw[:, 0:width], o[:],
                                    gb[:, g - b0, 0:width],
                                    start=st, stop=sp,
                                )
                                if with_cnt:
                                    nc.tensor.matmul(
                                        pwc[:], o[:],
                                        ones_sb[:], start=st, stop=sp,
                                    )
                            nc.vector.tensor_tensor(
                                out=acc_tile[:, w, 0:width],
                                in0=acc_tile[:, w, 0:width],
                                in1=pw[:, 0:width],
                                op=mybir.AluOpType.add,
                            )
                            if with_cnt:
                                nc.vector.tensor_tensor(
                                    out=acc_tile[:, w, width : width + 1],
                                    in0=acc_tile[:, w, width : width + 1],
                                    in1=pwc[:],
                                    op=mybir.AluOpType.add,
                                )

            # ================= conv1 aggregation =================
            aggregation((x_lo, x_hi), aggcnt, D, True)

            # ================= dense phase =================
            with (
                tc.tile_pool(name="dwork", bufs=3) as dwork,
                tc.tile_pool(name="dsmall", bufs=4) as dsmall,
                tc.tile_pool(name="dpsum", bufs=2, space="PSUM") as dpsum,
                tc.tile_pool(name="dpsum2", bufs=1, space="PSUM") as dpsum2,
                tc.tile_pool(name="ptpool", bufs=1) as ptpool,
            ):
                pt_sb = ptpool.tile([2, cp], F32)
                for n in range(nw):
                    # reciprocal of count (and save for conv2)
                    cmax = dsmall.tile([128, 1], F32, tag="cmax")
                    nc.vector.tensor_scalar(
                        out=cmax[:], in0=aggcnt[:, n, D : D + 1],
                        scalar1=1.0, scalar2=None, op0=mybir.AluOpType.max,
                    )
                    nc.vector.reciprocal(rc_sb[:, n, :], cmax[:])
                    mean_n = dwork.tile([128, D], F32, tag="mean")
                    nc.vector.tensor_scalar(
                        out=mean_n[:], in0=aggcnt[:, n, 0:D],
                        scalar1=rc_sb[:, n, :], scalar2=None,
                        op0=mybir.AluOpType.mult,
                    )
                    # transposes: x_loc chunk and mean chunk -> feature major
                    tp = dpsum.tile([D, 128], F32, tag="tp")
                    nc.tensor.transpose(tp[:], xloc_sb[:, n, :], ident_sb[:])
                    xto = dwork.tile([D + 1, 128], F32, tag="xto")
                    nc.scalar.activation(
                        xto[0:D, :], tp[:], mybir.ActivationFunctionType.Copy)
                    nc.vector.memset(xto[D : D + 1, :], 1.0)
                    tp2 = dpsum.tile([D, 128], F32, tag="tp")
                    nc.tensor.transpose(tp2[:], mean_n[:], ident_sb[:])
                    meant = dwork.tile([D, 128], F32, tag="meant")
                    nc.scalar.activation(
                        meant[:], tp2[:], mybir.ActivationFunctionType.Copy)

                    # x1 = x@Wc + bc + mean@W1l   [128, HID]
                    x1p = dpsum2.tile([128, HID], F32, tag="x1")
                    nc.tensor.matmul(x1p[:], xto[:], wcb_sb[:],
                                     start=True, stop=False)
                    nc.tensor.matmul(x1p[:], meant[:], w1l_sb[:],
                                     start=False, stop=True)

                    # LayerNorm + ELU (node-major, free-dim reductions)
                    mu = dsmall.tile([128, 1], F32, tag="mu")
                    nc.vector.reduce_sum(out=mu[:], in_=x1p[:], axis=mybir.AxisListType.X)
                    nc.vector.tensor_scalar(
                        out=mu[:], in0=mu[:], scalar1=1.0 / HID,
                        scalar2=None, op0=mybir.AluOpType.mult)
                    xc = dwork.tile([128, HID], F32, tag="xc")
                    nc.vector.tensor_scalar(
                        out=xc[:], in0=x1p[:], scalar1=mu[:], scalar2=None,
                        op0=mybir.AluOpType.subtract)
                    sq = dwork.tile([128, HID], F32, tag="sq")
                    var = dsmall.tile([128, 1], F32, tag="var")
                    nc.scalar.activation(
                        sq[:], xc[:], mybir.ActivationFunctionType.Square,
                        accum_out=var[:])
                    rstd = dsmall.tile([128, 1], F32, tag="rstd")
                    nc.vector.tensor_scalar(
                        out=rstd[:], in0=var[:], scalar1=1.0 / HID,
                        scalar2=LN_EPS, op0=mybir.AluOpType.mult,
                        op1=mybir.AluOpType.add)
                    nc.scalar.activation(
                        rstd[:], rstd[:], mybir.ActivationFunctionType.Sqrt)
                    nc.vector.reciprocal(rstd[:], rstd[:])
                    y = dwork.tile([128, HID], F32, tag="y")
                    nc.vector.tensor_scalar(
                        out=y[:], in0=xc[:], scalar1=rstd[:], scalar2=None,
                        op0=mybir.AluOpType.mult)
                    nc.vector.tensor_tensor(
                        out=y[:], in0=y[:], in1=gamma_sb[:],
                        op=mybir.AluOpType.mult)
                    nc.vector.tensor_tensor(
                        out=y[:], in0=y[:], in1=beta_sb[:],
                        op=mybir.AluOpType.add)
                    # ELU: h = max(y,0)-1 + exp(min(y,0))
                    m0 = dwork.tile([128, HID], F32, tag="m0")
                    nc.vector.tensor_scalar(
                        out=m0[:], in0=y[:], scalar1=0.0, scalar2=None,
                        op0=mybir.AluOpType.min)
                    ex = dwork.tile([128, HID], F32, tag="ex")
                    nc.scalar.activation(
                        ex[:], m0[:], mybir.ActivationFunctionType.Exp)
                    rm1 = dwork.tile([128, HID], F32, tag="rm1")
                    nc.vector.tensor_scalar(
                        out=rm1[:], in0=y[:], scalar1=0.0, scalar2=-1.0,
                        op0=mybir.AluOpType.max, op1=mybir.AluOpType.add)
                    nc.vector.tensor_tensor(
                        out=h_sb[:, n, :], in0=rm1[:], in1=ex[:],
                        op=mybir.AluOpType.add)

                    # p | r = h @ [W2l | W2r]
                    prp = dpsum2.tile([128, 2 * OUT], F32, tag="pr")
                    for hh in range(2):
                        tph = dpsum.tile([128, 128], F32, tag="tph")
                        nc.tensor.transpose(
                            tph[:], h_sb[:, n, 128 * hh : 128 * (hh + 1)],
                            ident_sb[:])
                        hts = dwork.tile([128, 128], F32, tag="hts")
                        nc.scalar.activation(
                            hts[:], tph[:],
                            mybir.ActivationFunctionType.Copy)
                        nc.tensor.matmul(
                            prp[:], hts[:],
                            w2lr_sb[:, 4 * hh : 4 * (hh + 1)],
                            start=(hh == 0), stop=(hh == 1))
                    nc.scalar.activation(
                        pr_sb[:, n, :], prp[:],
                        mybir.ActivationFunctionType.Copy)
                    # p^T into [2, cp]
                    ptp = dpsum2.tile([OUT, 128], F32, tag="ptp")
                    nc.tensor.transpose(
                        ptp[:], pr_sb[:, n, 0:OUT], ident_sb[:])
                    nc.scalar.activation(
                        pt_sb[:, 128 * n : 128 * (n + 1)], ptp[:],
                        mybir.ActivationFunctionType.Copy)

                # ================= p all-gather =================
                nc.sync.dma_start(out=pt_dram[:], in_=pt_sb[:])
            nc.gpsimd.collective_compute(
                "AllGather",
                mybir.AluOpType.bypass,
                replica_groups=[list(range(N_CORES))],
                ins=[pt_dram.opt()],
                outs=[ptall_dram.opt()],
            )

            # ================= build pb2 (replicated p table) =================
            n_glob = N_CORES * nw  # global 128-node chunks
            stage_n = 14 if n_glob % 14 == 0 and (n_glob // 2) % 14 == 0 else 1
            half_rows_chunks = half // 128
            with (
                tc.tile_pool(name="bstage", bufs=2) as bstage,
                tc.tile_pool(name="bpt", bufs=2) as bpt,
                tc.tile_pool(name="bpsum", bufs=2, space="PSUM") as bpsum,
            ):
                stage = None
                ptb = None
                for j in range(n_glob):
                    c = j // nw
                    jw = j % nw
                    if jw == 0:
                        ptb = bpt.tile([2, cp], F32, tag="ptb", name="ptb")
                        nc.sync.dma_start(
                            out=ptb[:], in_=ptall_dram[2 * c : 2 * c + 2, :])
                    pp = bpsum.tile([128, D], F32, tag="pb2p")
                    nc.tensor.matmul(
                        pp[:],
                        ptb[:, 128 * jw : 128 * (jw + 1)],
                        i2_sb[:], start=True, stop=True)
                    if j % stage_n == 0:
                        stage = bstage.tile([128, stage_n, D], F32,
                                            tag="stage")
                    nc.scalar.activation(
                        stage[:, j % stage_n, :], pp[:],
                        mybir.ActivationFunctionType.Copy)
                    if j % stage_n == stage_n - 1:
                        j0 = j - stage_n + 1
                        r0 = j0 * 128  # global row
                        if r0 < half:
                            dst = pb2_lo[r0 : r0 + stage_n * 128, :]
                        else:
                            dst = pb2_hi[r0 - half : r0 - half
                                         + stage_n * 128, :]
                        nc.sync.dma_start(
                            out=dst.rearrange("(s p) d -> p s d", p=128),
                            in_=stage[:])

            # ================= conv2 aggregation =================
            aggregation((pb2_lo, pb2_hi), agg2, OUT, False)

            # ================= output =================
            with tc.tile_pool(name="fsmall", bufs=4) as fsmall:
                for n in range(nw):
                    t = fsmall.tile([128, OUT], F32, tag="fo")
                    nc.vector.tensor_scalar(
                        out=t[:], in0=agg2[:, n, :], scalar1=rc_sb[:, n, :],
                        scalar2=None, op0=mybir.AluOpType.mult)
                    nc.vector.tensor_tensor(
                        out=t[:], in0=t[:], in1=pr_sb[:, n, OUT : 2 * OUT],
                        op=mybir.AluOpType.add)
                    nc.vector.tensor_tensor(
                        out=out_sb[:, n, :], in0=t[:], in1=b2_sb[:],
                        op=mybir.AluOpType.add)
            nc.sync.dma_start(
                out=out_d.rearrange("(w p) c -> p w c", p=128),
                in_=out_sb[:])

    nc.compile()
    return nc


def make_inputs(plan, x, W1l, W1r, b1, Wskip, bskip, gamma, beta, W2l, W2r,
                b2, n_nodes):
    cp, half, npad = plan["cp"], plan["half"], plan["npad"]
    xp = np.zeros((npad, D), np.float32)
    xp[:n_nodes] = np.asarray(x, np.float32)
    wc = np.asarray(W1r, np.float32) + np.asarray(Wskip, np.float32)
    bc = np.asarray(b1, np.float32) + np.asarray(bskip, np.float32)
    wcb = np.concatenate([wc, bc[None, :]], axis=0)
    w2lr_full = np.concatenate(
        [np.asarray(W2l, np.float32), np.asarray(W2r, np.float32)], axis=1
    )  # [HID, 4]
    w2lr = (
        w2lr_full.reshape(2, 128, 2 * OUT).transpose(1, 0, 2)
        .reshape(128, 2 * 2 * OUT).copy()
    )
    iota = np.tile(np.arange(128, dtype=np.float32)[None, :], (128, 1))
    ident = np.eye(128, dtype=np.float32)
    i2 = np.zeros((2, D), np.float32)
    i2[0, 0::2] = 1.0
    i2[1, 1::2] = 1.0
    gamma_bc = np.tile(np.asarray(gamma, np.float32)[None, :], (128, 1))
    beta_bc = np.tile(np.asarray(beta, np.float32)[None, :], (128, 1))
    b2_bc = np.tile(np.asarray(b2, np.float32)[None, :], (128, 1))

    common = dict(
        x_lo=xp[:half].copy(), x_hi=xp[half:].copy(),
        iota=iota, ident=ident,
        wcb=wcb, w1l=np.asarray(W1l, np.float32), w2lr=w2lr,
        gamma_bc=gamma_bc, beta_bc=beta_bc, b2_bc=b2_bc, i2=i2,
    )
    in_maps = []
    for c in range(N_CORES):
        m = dict(common)
        m["x_loc"] = xp[cp * c : cp * (c + 1)].copy()
        m["gidx"] = plan["gidx_tile"][c]
        m["dstf"] = plan["dstf_tile"][c]
        in_maps.append(m)
    return in_maps


_CACHE = {}


def _get_compiled(edge_index, n_nodes):
    key = (edge_index.tobytes()[:512], edge_index.shape, n_nodes)
    if key not in _CACHE:
        plan = make_plan(edge_index, n_nodes)
        nc = build_program(plan)
        _CACHE[key] = (plan, nc)
    return _CACHE[key]


def run(inputs, trace=False):
    x = np.asarray(inputs["x"], np.float32)
    edge_index = np.asarray(inputs["edge_index"], np.int32)
    n_nodes = x.shape[0]
    plan, nc = _get_compiled(edge_index, n_nodes)
    in_maps = make_inputs(
        plan, x, inputs["W1l"], inputs["W1r"], inputs["b1"], inputs["Wskip"],
        inputs["bskip"], inputs["gamma"], inputs["beta"], inputs["W2l"],
        inputs["W2r"], inputs["b2"], n_nodes)
    res = run_bass_kernel_spmd(
        nc, in_maps, list(range(N_CORES)), trace=trace)
    cp = plan["cp"]
    out = np.empty((n_nodes, OUT), np.float32)
    for c in range(N_CORES):
        lo = cp * c
        hi = min(cp * (c + 1), n_nodes)
        out[lo:hi] = res.results[c]["out"][0 : hi - lo]
    return out, res


def kernel(**inputs) -> np.ndarray:
    out, _ = run(inputs)
    return out



# revision 11
# speedup vs baseline: 1.2265x; 1.2265x over previous
"""GeneSAGE (2-layer GraphSAGE + skip + LayerNorm + ELU) on 8 Trainium2 cores.

Strategy: edge-parallel by *destination range*. Core c owns nodes
[CP*c, CP*(c+1)) with CP=6272 (=49*128). Edges are bucketed host-side by
(dst-core, src-half, dst-window) for conv1 and (dst-core, dst-window) for
conv2, padded to 128-edge chunks with a chunk structure common to all 8
cores (SPMD: one program). Per chunk: one-hot(dst) built on DVE, segment
sum done as one-hot matmuls (float32r) accumulating in PSUM per 128-node
window. Conv1 gathers x rows (256B) from HBM via dma_gather. Node degrees
(and their reciprocals) are precomputed on the host. The dense phase
(mean/linear/LN/ELU/p,r) is interleaved per-window under conv1's second
edge stream. Conv2 gathers from a 32-node-packed p table ([npad, 2] f32
viewed as [npad/32, 64], built by a tiny 50KB-per-core AllGather of the
locally computed p) and extracts the 8B pair per edge with one fused
scalar_tensor_tensor, accumulating a [128, 64] PSUM per window that is
tree-reduced at the end.
"""

import numpy as np

import concourse.mybir as mybir
from concourse import bacc, bass, tile
from concourse.bass_utils import run_bass_kernel_spmd

F32 = mybir.dt.float32
F32R = mybir.dt.float32r
I16 = mybir.dt.int16

N_CORES = 8
D = 64          # input feature dim
HID = 256
OUT = 2
LN_EPS = 1e-5
BATCH_CHUNKS = 32   # chunks per dma_gather call
USE_F32R = False


def _mm_cast(ap):
    return ap.bitcast(F32R) if USE_F32R else ap


def make_plan(edge_index: np.ndarray, n_nodes: int):
    """Host-side edge bucketing + degree precompute."""
    cp = int(np.ceil(n_nodes / (N_CORES * 128))) * 128
    nw = cp // 128
    npad = N_CORES * cp
    half = npad // 2
    assert half <= 32768, "int16 gather index limit"

    src = edge_index[0].astype(np.int64)
    dst = edge_index[1].astype(np.int64)
    E = src.shape[0]

    # degrees -> reciprocal of count per node, [cores, 128, nw]
    deg = np.bincount(dst, minlength=npad).astype(np.float64)
    rc = (1.0 / np.maximum(deg, 1.0)).astype(np.float32)
    rc_tile = rc.reshape(N_CORES, nw, 128).transpose(0, 2, 1).copy()

    core = dst // cp
    win = (dst % cp) // 128

    def bucket(streams, gidx_vals, extra_vals=None):
        """streams: per-edge stream id (0..S-1); returns plan piece."""
        S = int(streams.max()) + 1 if E else 1
        ngrp = S * nw
        key = (core * S + streams) * nw + win
        order = np.argsort(key, kind="stable")
        counts = np.bincount(key, minlength=N_CORES * ngrp).reshape(
            N_CORES, S, nw)
        nchunks = -(-counts.max(axis=0) // 128)  # [S, nw]
        off = np.zeros((S, nw), np.int64)
        running = 0
        for s in range(S):
            for w in range(nw):
                off[s, w] = running
                running += nchunks[s, w]
        c_total = int(running)
        e_slots = c_total * 128

        sk = key[order]
        grp_start = np.searchsorted(sk, np.arange(N_CORES * ngrp))
        rank = np.arange(E) - grp_start[sk]
        s_of = (sk // nw) % S
        w_of = sk % nw
        c_of = sk // ngrp
        slot = off[s_of, w_of] * 128 + rank

        gidx = np.zeros((N_CORES, e_slots), np.int16)
        dstf = np.full((N_CORES, e_slots), -1.0, np.float32)
        gidx[c_of, slot] = gidx_vals[order].astype(np.int16)
        dstf[c_of, slot] = (dst[order] % cp - w_of * 128).astype(np.float32)
        extra = None
        if extra_vals is not None:
            extra = np.full((N_CORES, e_slots), -1.0, np.float32)
            extra[c_of, slot] = extra_vals[order].astype(np.float32)

        a = gidx.reshape(N_CORES, e_slots // 16, 16).transpose(0, 2, 1)
        gidx_tile = np.tile(a, (1, 8, 1)).copy()  # [c, 128, J]
        dstf_tile = dstf.reshape(N_CORES, c_total, 128).transpose(0, 2, 1).copy()
        extra_tile = None
        if extra is not None:
            extra_tile = extra.reshape(N_CORES, c_total, 128).transpose(
                0, 2, 1).copy()

        sched = []
        for s in range(S):
            rows = []
            for w in range(nw):
                n = int(nchunks[s, w])
                first = int(off[s, w])
                rows.append((w, first, first + n - 1) if n else (w, -1, -2))
            sched.append(rows)
        return dict(c_total=c_total, sched=sched, gidx_tile=gidx_tile,
                    dstf_tile=dstf_tile, extra_tile=extra_tile)

    # conv1: 2 streams by src half, gather idx = src - s*half
    stream1 = (src >= half).astype(np.int64)
    p1 = bucket(stream1, src - stream1 * half)
    # conv2: single stream, gather idx = src >> 5 (32-packed p table),
    # extra = src & 31 (pair slot within the 256B row)
    p2 = bucket(np.zeros(E, np.int64), src >> 5, src & 31)

    return dict(cp=cp, nw=nw, npad=npad, half=half, p1=p1, p2=p2,
                rc_tile=rc_tile)


def build_program(plan):
    cp, nw, half, npad = plan["cp"], plan["nw"], plan["half"], plan["npad"]
    p1, p2 = plan["p1"], plan["p2"]
    c1, c2 = p1["c_total"], p2["c_total"]
    J1, J2 = c1 * 8, c2 * 8

    nc = bacc.Bacc("TRN2", target_bir_lowering=False, debug=False,
                   num_devices=N_CORES)

    def inp(name, shape, dt=F32):
        return nc.dram_tensor(name, shape, dt, kind="ExternalInput").ap()

    x_lo = inp("x_lo", [half, D])
    x_hi = inp("x_hi", [half, D])
    xt_d = inp("xt", [D + 1, cp])          # x^T with ones row
    gidx1_d = inp("gidx1", [128, J1], I16)
    dstf1_d = inp("dstf1", [128, c1])
    gidx2_d = inp("gidx2", [128, J2], I16)
    dstf2_d = inp("dstf2", [128, c2])
    kf2_d = inp("kf2", [128, c2])
    rc_d = inp("rc", [128, nw])
    iota_d = inp("iota", [128, 128])
    iota32_d = inp("iota32", [128, D])     # floor(j/2) pattern
    ident_d = inp("ident", [128, 128])
    wcb_d = inp("wcb", [D + 1, HID])       # [W1r+Wskip; b1+bskip]
    w1l_d = inp("w1l", [D, HID])
    w2lr_d = inp("w2lr", [128, 2 * 2 * OUT])  # halves of [W2l|W2r] packed
    gamma_d = inp("gamma_bc", [128, HID])
    beta_d = inp("beta_bc", [128, HID])
    b2_d = inp("b2_bc", [128, OUT])
    out_d = nc.dram_tensor("out", [cp, OUT], F32, kind="ExternalOutput").ap()

    with tile.TileContext(nc) as tc:
        with (
            tc.tile_pool(name="res", bufs=1) as res,
            tc.tile_pool(name="dram", bufs=1, space="DRAM") as dram,
        ):
            # ---- resident tiles / constants
            gidx1_sb = res.tile([128, J1], I16)
            nc.sync.dma_start(out=gidx1_sb[:], in_=gidx1_d[:])
            dstf1_sb = res.tile([128, c1], F32)
            nc.sync.dma_start(out=dstf1_sb[:], in_=dstf1_d[:])
            gidx2_sb = res.tile([128, J2], I16)
            nc.sync.dma_start(out=gidx2_sb[:], in_=gidx2_d[:])
            dstf2_sb = res.tile([128, c2], F32)
            nc.sync.dma_start(out=dstf2_sb[:], in_=dstf2_d[:])
            kf2_sb = res.tile([128, c2], F32)
            nc.sync.dma_start(out=kf2_sb[:], in_=kf2_d[:])
            rc_sb = res.tile([128, nw], F32)
            nc.sync.dma_start(out=rc_sb[:], in_=rc_d[:])
            iota_sb = res.tile([128, 128], F32)
            nc.sync.dma_start(out=iota_sb[:], in_=iota_d[:])
            iota32_sb = res.tile([128, D], F32)
            nc.sync.dma_start(out=iota32_sb[:], in_=iota32_d[:])
            ident_sb = res.tile([128, 128], F32)
            nc.sync.dma_start(out=ident_sb[:], in_=ident_d[:])
            xt_sb = res.tile([D + 1, cp], F32)
            nc.sync.dma_start(out=xt_sb[:], in_=xt_d[:])
            wcb_sb = res.tile([D + 1, HID], F32)
            nc.sync.dma_start(out=wcb_sb[:], in_=wcb_d[:])
            w1l_sb = res.tile([D, HID], F32)
            nc.sync.dma_start(out=w1l_sb[:], in_=w1l_d[:])
            w2lr_sb = res.tile([128, 2 * 2 * OUT], F32)
            nc.sync.dma_start(out=w2lr_sb[:], in_=w2lr_d[:])
            gamma_sb = res.tile([128, HID], F32)
            nc.sync.dma_start(out=gamma_sb[:], in_=gamma_d[:])
            beta_sb = res.tile([128, HID], F32)
            nc.sync.dma_start(out=beta_sb[:], in_=beta_d[:])
            b2_sb = res.tile([128, OUT], F32)
            nc.sync.dma_start(out=b2_sb[:], in_=b2_d[:])

            agg = res.tile([128, nw, D], F32)
            nc.vector.memset(agg[:], 0.0)
            pr_sb = res.tile([128, nw, 2 * OUT], F32)
            out_sb = res.tile([128, nw, OUT], F32)

            pk_local = dram.tile([cp, OUT], F32)
            pk_all = dram.tile([npad // 32, D], F32)

            # =========== conv1 aggregation + interleaved dense ===========
            sched1 = p1["sched"]
            with (
                tc.tile_pool(name="gpool", bufs=3) as gpool,
                tc.tile_pool(name="opool", bufs=4) as opool,
                tc.tile_pool(name="pwpool", bufs=2, space="PSUM") as pwp,
                tc.tile_pool(name="dwork", bufs=3) as dwork,
                tc.tile_pool(name="dsmall", bufs=4) as dsmall,
                tc.tile_pool(name="dpsum", bufs=2, space="PSUM") as dpsum,
                tc.tile_pool(name="dpsum2", bufs=1, space="PSUM") as dpsum2,
            ):
                # issue all conv1 gathers (both streams), batched
                gbufs = {}
                for s, table in ((0, x_lo), (1, x_hi)):
                    rows = [r for r in sched1[s] if r[1] >= 0]
                    if not rows:
                        continue
                    c0 = rows[0][1]
                    cend = rows[-1][2] + 1
                    for b0 in range(c0, cend, BATCH_CHUNKS):
                        b1 = min(b0 + BATCH_CHUNKS, cend)
                        g = gpool.tile([128, BATCH_CHUNKS, D], F32, tag="gbuf")
                        n_idx = (b1 - b0) * 128
                        nc.gpsimd.dma_gather(
                            out_ap=g[:, 0 : b1 - b0, :],
                            in_ap=table,
                            idxs_ap=gidx1_sb[:, b0 * 8 : b1 * 8],
                            num_idxs=n_idx,
                            num_idxs_reg=n_idx,
                            elem_size=D,
                            single_packet=False,
                        )
                        gbufs[s, b0] = g

                def accumulate(s, w, first, last, dstf_sb, gb_of):
                    """one-hot matmuls for chunks [first..last] into PSUM,
                    then add into agg[:, w]."""
                    if first > last:
                        return
                    pw = pwp.tile([128, D], F32, tag="pw")
                    for g in range(first, last + 1):
                        gb, col = gb_of(g)
                        o = opool.tile([128, 128], F32, tag="O")
                        nc.vector.tensor_scalar(
                            out=o[:], in0=iota_sb[:],
                            scalar1=dstf_sb[:, g : g + 1], scalar2=None,
                            op0=mybir.AluOpType.is_equal,
                        )
                        nc.tensor.matmul(
                            pw[:], _mm_cast(o[:]), _mm_cast(gb[:, col, :]),
                            start=(g == first), stop=(g == last),
                        )
                    nc.vector.tensor_tensor(
                        out=agg[:, w, :], in0=agg[:, w, :], in1=pw[:],
                        op=mybir.AluOpType.add,
                    )

                def gb_of1(s):
                    rows = [r for r in sched1[s] if r[1] >= 0]
                    c0 = rows[0][1]

                    def f(g):
                        b0 = c0 + ((g - c0) // BATCH_CHUNKS) * BATCH_CHUNKS
                        return gbufs[s, b0], g - b0
                    return f

                # stream 0: plain accumulation
                if any(r[1] >= 0 for r in sched1[0]):
                    f0 = gb_of1(0)
                    for w, first, last in sched1[0]:
                        accumulate(0, w, first, last, dstf1_sb, f0)

                # stream 1: accumulate, then dense phase for that window
                f1 = gb_of1(1) if any(r[1] >= 0 for r in sched1[1]) else None
                for w, first, last in sched1[1]:
                    if f1 is not None:
                        accumulate(1, w, first, last, dstf1_sb, f1)

                    # ---------- dense phase for window w ----------
                    mean_n = dwork.tile([128, D], F32, tag="mean")
                    nc.vector.tensor_scalar(
                        out=mean_n[:], in0=agg[:, w, :],
                        scalar1=rc_sb[:, w : w + 1], scalar2=None,
                        op0=mybir.AluOpType.mult,
                    )
                    tp2 = dpsum.tile([D, 128], F32, tag="tp")
                    nc.tensor.transpose(tp2[:], mean_n[:], ident_sb[:])
                    meant = dwork.tile([D, 128], F32, tag="meant")
                    nc.scalar.activation(
                        meant[:], tp2[:], mybir.ActivationFunctionType.Copy)

                    # x1 = x@Wc + bc + mean@W1l   [128, HID]
                    x1p = dpsum2.tile([128, HID], F32, tag="x1")
                    nc.tensor.matmul(
                        x1p[:], xt_sb[:, 128 * w : 128 * (w + 1)], wcb_sb[:],
                        start=True, stop=False)
                    nc.tensor.matmul(x1p[:], meant[:], w1l_sb[:],
                                     start=False, stop=True)

                    # LayerNorm + ELU
                    mu = dsmall.tile([128, 1], F32, tag="mu")
                    nc.vector.reduce_sum(out=mu[:], in_=x1p[:],
                                         axis=mybir.AxisListType.X)
                    nc.vector.tensor_scalar(
                        out=mu[:], in0=mu[:], scalar1=1.0 / HID,
                        scalar2=None, op0=mybir.AluOpType.mult)
                    xc = dwork.tile([128, HID], F32, tag="xc")
                    nc.vector.tensor_scalar(
                        out=xc[:], in0=x1p[:], scalar1=mu[:], scalar2=None,
                        op0=mybir.AluOpType.subtract)
                    sq = dwork.tile([128, HID], F32, tag="sq")
                    var = dsmall.tile([128, 1], F32, tag="var")
                    nc.scalar.activation(
                        sq[:], xc[:], mybir.ActivationFunctionType.Square,
                        accum_out=var[:])
                    rstd = dsmall.tile([128, 1], F32, tag="rstd")
                    nc.vector.tensor_scalar(
                        out=rstd[:], in0=var[:], scalar1=1.0 / HID,
                        scalar2=LN_EPS, op0=mybir.AluOpType.mult,
                        op1=mybir.AluOpType.add)
                    nc.scalar.activation(
                        rstd[:], rstd[:], mybir.ActivationFunctionType.Sqrt)
                    nc.vector.reciprocal(rstd[:], rstd[:])
                    y = dwork.tile([128, HID], F32, tag="y")
                    nc.vector.scalar_tensor_tensor(
                        out=y[:], in0=xc[:], scalar=rstd[:], in1=gamma_sb[:],
                        op0=mybir.AluOpType.mult, op1=mybir.AluOpType.mult)
                    nc.vector.tensor_tensor(
                        out=y[:], in0=y[:], in1=beta_sb[:],
                        op=mybir.AluOpType.add)
                    # ELU: h = (max(y,0)-1) + exp(min(y,0))
                    m0 = dwork.tile([128, HID], F32, tag="m0")
                    nc.vector.tensor_scalar(
                        out=m0[:], in0=y[:], scalar1=0.0, scalar2=None,
                        op0=mybir.AluOpType.min)
                    ex = dwork.tile([128, HID], F32, tag="ex")
                    nc.scalar.activation(
                        ex[:], m0[:], mybir.ActivationFunctionType.Exp)
                    rm1 = dwork.tile([128, HID], F32, tag="rm1")
                    nc.vector.tensor_scalar(
                        out=rm1[:], in0=y[:], scalar1=0.0, scalar2=-1.0,
                        op0=mybir.AluOpType.max, op1=mybir.AluOpType.add)
                    h = dwork.tile([128, HID], F32, tag="h")
                    nc.vector.tensor_tensor(
                        out=h[:], in0=rm1[:], in1=ex[:],
                        op=mybir.AluOpType.add)

                    # p | r = h @ [W2l | W2r]
                    prp = dpsum2.tile([128, 2 * OUT], F32, tag="pr")
                    for hh in range(2):
                        tph = dpsum.tile([128, 128], F32, tag="tph")
                        nc.tensor.transpose(
                            tph[:], h[:, 128 * hh : 128 * (hh + 1)],
                            ident_sb[:])
                        hts = dwork.tile([128, 128], F32, tag="hts")
                        nc.scalar.activation(
                            hts[:], tph[:], mybir.ActivationFunctionType.Copy)
                        nc.tensor.matmul(
                            prp[:], hts[:],
                            w2lr_sb[:, 4 * hh : 4 * (hh + 1)],
                            start=(hh == 0), stop=(hh == 1))
                    nc.scalar.activation(
                        pr_sb[:, w, :], prp[:],
                        mybir.ActivationFunctionType.Copy)

                # p (node-major pairs) -> DRAM for the allgather
                nc.sync.dma_start(
                    out=pk_local.rearrange("(w p) c -> p w c", p=128),
                    in_=pr_sb[:, :, 0:OUT])

            # ================= p all-gather (50KB per core) =================
            nc.gpsimd.collective_compute(
                "AllGather",
                mybir.AluOpType.bypass,
                replica_groups=[list(range(N_CORES))],
                ins=[pk_local.opt()],
                outs=[pk_all.opt()],
            )

            # ================= conv2 aggregation =================
            sched2 = p2["sched"][0]
            with (
                tc.tile_pool(name="g2pool", bufs=3) as g2pool,
                tc.tile_pool(name="o2pool", bufs=4) as o2pool,
                tc.tile_pool(name="m2pool", bufs=4) as m2pool,
                tc.tile_pool(name="pw2pool", bufs=2, space="PSUM") as pw2p,
                tc.tile_pool(name="fwork", bufs=4) as fwork,
            ):
                rows = [r for r in sched2 if r[1] >= 0]
                c0 = rows[0][1]
                cend = rows[-1][2] + 1
                g2bufs = {}
                for b0 in range(c0, cend, BATCH_CHUNKS):
                    b1 = min(b0 + BATCH_CHUNKS, cend)
                    g = g2pool.tile([128, BATCH_CHUNKS, D], F32, tag="g2buf")
                    n_idx = (b1 - b0) * 128
                    nc.gpsimd.dma_gather(
                        out_ap=g[:, 0 : b1 - b0, :],
                        in_ap=pk_all,
                        idxs_ap=gidx2_sb[:, b0 * 8 : b1 * 8],
                        num_idxs=n_idx,
                        num_idxs_reg=n_idx,
                        elem_size=D,
                        single_packet=False,
                    )
                    g2bufs[b0] = g

                for w, first, last in sched2:
                    if first > last:
                        nc.vector.tensor_tensor(
                            out=out_sb[:, w, :],
                            in0=pr_sb[:, w, OUT : 2 * OUT], in1=b2_sb[:],
                            op=mybir.AluOpType.add)
                        continue
                    pw2 = pw2p.tile([128, D], F32, tag="pw2")
                    for g in range(first, last + 1):
                        b0 = c0 + ((g - c0) // BATCH_CHUNKS) * BATCH_CHUNKS
                        gb = g2bufs[b0]
                        # masked row: (iota32 == k) * gathered
                        mt = m2pool.tile([128, D], F32, tag="mt")
                        nc.vector.scalar_tensor_tensor(
                            out=mt[:], in0=iota32_sb[:],
                            scalar=kf2_sb[:, g : g + 1],
                            in1=gb[:, g - b0, :],
                            op0=mybir.AluOpType.is_equal,
                            op1=mybir.AluOpType.mult)
                        o = o2pool.tile([128, 128], F32, tag="O2")
                        nc.vector.tensor_scalar(
                            out=o[:], in0=iota_sb[:],
                            scalar1=dstf2_sb[:, g : g + 1], scalar2=None,
                            op0=mybir.AluOpType.is_equal,
                        )
                        nc.tensor.matmul(
                            pw2[:], _mm_cast(o[:]), _mm_cast(mt[:]),
                            start=(g == first), stop=(g == last),
                        )
                    # tree-reduce the 32 packed slots: [128,64] -> [128,2]
                    s64 = fwork.tile([128, D], F32, tag="s64")
                    nc.scalar.activation(
                        s64[:], pw2[:], mybir.ActivationFunctionType.Copy)
                    t32 = fwork.tile([128, 32], F32, tag="t32")
                    nc.vector.tensor_tensor(
                        out=t32[:], in0=s64[:, 0:32], in1=s64[:, 32:64],
                        op=mybir.AluOpType.add)
                    t16 = fwork.tile([128, 16], F32, tag="t16")
                    nc.vector.tensor_tensor(
                        out=t16[:], in0=t32[:, 0:16], in1=t32[:, 16:32],
                        op=mybir.AluOpType.add)
                    t8 = fwork.tile([128, 8], F32, tag="t8")
                    nc.vector.tensor_tensor(
                        out=t8[:], in0=t16[:, 0:8], in1=t16[:, 8:16],
                        op=mybir.AluOpType.add)
                    t4 = fwork.tile([128, 4], F32, tag="t4")
                    nc.vector.tensor_tensor(
                        out=t4[:], in0=t8[:, 0:4], in1=t8[:, 4:8],
                        op=mybir.AluOpType.add)
                    t2 = fwork.tile([128, 2], F32, tag="t2")
                    nc.vector.tensor_tensor(
                        out=t2[:], in0=t4[:, 0:2], in1=t4[:, 2:4],
                        op=mybir.AluOpType.add)
                    # out = t2 * rc + r + b2
                    t = fwork.tile([128, OUT], F32, tag="fo")
                    nc.vector.tensor_scalar(
                        out=t[:], in0=t2[:], scalar1=rc_sb[:, w : w + 1],
                        scalar2=None, op0=mybir.AluOpType.mult)
                    nc.vector.tensor_tensor(
                        out=t[:], in0=t[:], in1=pr_sb[:, w, OUT : 2 * OUT],
                        op=mybir.AluOpType.add)
                    nc.vector.tensor_tensor(
                        out=out_sb[:, w, :], in0=t[:], in1=b2_sb[:],
                        op=mybir.AluOpType.add)

            nc.sync.dma_start(
                out=out_d.rearrange("(w p) c -> p w c", p=128),
                in_=out_sb[:])

    nc.compile()
    return nc


def make_inputs(plan, x, W1l, W1r, b1, Wskip, bskip, gamma, beta, W2l, W2r,
                b2, n_nodes):
    cp, half, npad, nw = plan["cp"], plan["half"], plan["npad"], plan["nw"]
    xp = np.zeros((npad, D), np.float32)
    xp[:n_nodes] = np.asarray(x, np.float32)
    wc = np.asarray(W1r, np.float32) + np.asarray(Wskip, np.float32)
    bc = np.asarray(b1, np.float32) + np.asarray(bskip, np.float32)
    wcb = np.concatenate([wc, bc[None, :]], axis=0)
    w2lr_full = np.concatenate(
        [np.asarray(W2l, np.float32), np.asarray(W2r, np.float32)], axis=1
    )  # [HID, 4]
    w2lr = (
        w2lr_full.reshape(2, 128, 2 * OUT).transpose(1, 0, 2)
        .reshape(128, 2 * 2 * OUT).copy()
    )
    iota = np.tile(np.arange(128, dtype=np.float32)[None, :], (128, 1))
    iota32 = np.tile(
        (np.arange(D, dtype=np.float32) // 2)[None, :], (128, 1))
    ident = np.eye(128, dtype=np.float32)
    gamma_bc = np.tile(np.asarray(gamma, np.float32)[None, :], (128, 1))
    beta_bc = np.tile(np.asarray(beta, np.float32)[None, :], (128, 1))
    b2_bc = np.tile(np.asarray(b2, np.float32)[None, :], (128, 1))

    common = dict(
        x_lo=xp[:half].copy(), x_hi=xp[half:].copy(),
        iota=iota, iota32=iota32, ident=ident,
        wcb=wcb, w1l=np.asarray(W1l, np.float32), w2lr=w2lr,
        gamma_bc=gamma_bc, beta_bc=beta_bc, b2_bc=b2_bc,
    )
    in_maps = []
    for c in range(N_CORES):
        m = dict(common)
        xc_loc = xp[cp * c : cp * (c + 1)]
        xt = np.empty((D + 1, cp), np.float32)
        xt[0:D] = xc_loc.T
        xt[D] = 1.0
        m["xt"] = xt
        m["gidx1"] = plan["p1"]["gidx_tile"][c]
        m["dstf1"] = plan["p1"]["dstf_tile"][c]
        m["gidx2"] = plan["p2"]["gidx_tile"][c]
        m["dstf2"] = plan["p2"]["dstf_tile"][c]
        m["kf2"] = plan["p2"]["extra_tile"][c]
        m["rc"] = plan["rc_tile"][c]
        in_maps.append(m)
    return in_maps


_CACHE = {}


def _get_compiled(edge_index, n_nodes):
    key = (edge_index.tobytes()[:512], edge_index.shape, n_nodes)
    if key not in _CACHE:
        plan = make_plan(edge_index, n_nodes)
        nc = build_program(plan)
        _CACHE[key] = (plan, nc)
    return _CACHE[key]


def run(inputs, trace=False):
    x = np.asarray(inputs["x"], np.float32)
    edge_index = np.asarray(inputs["edge_index"], np.int32)
    n_nodes = x.shape[0]
    plan, nc = _get_compiled(edge_index, n_nodes)
    in_maps = make_inputs(
        plan, x, inputs["W1l"], inputs["W1r"], inputs["b1"], inputs["Wskip"],
        inputs["bskip"], inputs["gamma"], inputs["beta"], inputs["W2l"],
        inputs["W2r"], inputs["b2"], n_nodes)
    res = run_bass_kernel_spmd(
        nc, in_maps, list(range(N_CORES)), trace=trace)
    cp = plan["cp"]
    out = np.empty((n_nodes, OUT), np.float32)
    for c in range(N_CORES):
        lo = cp * c
        hi = min(cp * (c + 1), n_nodes)
        out[lo:hi] = res.results[c]["out"][0 : hi - lo]
    return out, res


def kernel(**inputs) -> np.ndarray:
    out, _ = run(inputs)
    return out


# revision 16
# speedup vs baseline: 1.2537x; 1.0222x over previous
"""GeneSAGE (2-layer GraphSAGE + skip + LayerNorm + ELU) on 8 Trainium2 cores.

Strategy: edge-parallel by *destination range*. Core c owns nodes
[CP*c, CP*(c+1)) with CP=6272 (=49*128). Edges are bucketed host-side by
(dst-core, src-half, dst-window) for conv1 and (dst-core, dst-window) for
conv2, padded to 128-edge chunks with a chunk structure common to all 8
cores (SPMD: one program). Per chunk: one-hot(dst) built on DVE, segment
sum done as one-hot matmuls (float32r) accumulating in PSUM per 128-node
window. Conv1 gathers x rows (256B) from HBM via dma_gather. Node degrees
(and their reciprocals) are precomputed on the host. The dense phase
(mean/linear/LN/ELU/p,r) is interleaved per-window under conv1's second
edge stream. Conv2 gathers from a 32-node-packed p table ([npad, 2] f32
viewed as [npad/32, 64], built by a tiny 50KB-per-core AllGather of the
locally computed p) and extracts the 8B pair per edge with one fused
scalar_tensor_tensor, accumulating a [128, 64] PSUM per window that is
tree-reduced at the end.
"""

import numpy as np

import concourse.mybir as mybir
from concourse import bacc, bass, tile
from concourse.bass_utils import run_bass_kernel_spmd

F32 = mybir.dt.float32
F32R = mybir.dt.float32r
I16 = mybir.dt.int16

N_CORES = 8
D = 64          # input feature dim
HID = 256
OUT = 2
LN_EPS = 1e-5
BATCH_CHUNKS = 32   # chunks per dma_gather call
USE_F32R = False
N_SWDGE_Q = 1


def _mm_cast(ap):
    return ap.bitcast(F32R) if USE_F32R else ap


def make_plan(edge_index: np.ndarray, n_nodes: int):
    """Host-side edge bucketing + degree precompute."""
    cp = int(np.ceil(n_nodes / (N_CORES * 128))) * 128
    nw = cp // 128
    npad = N_CORES * cp
    half = npad // 2
    assert half <= 32768, "int16 gather index limit"

    src = edge_index[0].astype(np.int64)
    dst = edge_index[1].astype(np.int64)
    E = src.shape[0]

    # degrees -> reciprocal of count per node, [cores, 128, nw]
    deg = np.bincount(dst, minlength=npad).astype(np.float64)
    rc = (1.0 / np.maximum(deg, 1.0)).astype(np.float32)
    rc_tile = rc.reshape(N_CORES, nw, 128).transpose(0, 2, 1).copy()

    core = dst // cp
    win = (dst % cp) // 128

    def bucket(streams, gidx_vals, extra_vals=None):
        """streams: per-edge stream id (0..S-1); returns plan piece."""
        S = int(streams.max()) + 1 if E else 1
        ngrp = S * nw
        key = (core * S + streams) * nw + win
        order = np.argsort(key, kind="stable")
        counts = np.bincount(key, minlength=N_CORES * ngrp).reshape(
            N_CORES, S, nw)
        nchunks = -(-counts.max(axis=0) // 128)  # [S, nw]
        off = np.zeros((S, nw), np.int64)
        running = 0
        for s in range(S):
            for w in range(nw):
                off[s, w] = running
                running += nchunks[s, w]
        c_total = int(running)
        e_slots = c_total * 128

        sk = key[order]
        grp_start = np.searchsorted(sk, np.arange(N_CORES * ngrp))
        rank = np.arange(E) - grp_start[sk]
        s_of = (sk // nw) % S
        w_of = sk % nw
        c_of = sk // ngrp
        slot = off[s_of, w_of] * 128 + rank

        gidx = np.zeros((N_CORES, e_slots), np.int16)
        dstf = np.full((N_CORES, e_slots), -1.0, np.float32)
        gidx[c_of, slot] = gidx_vals[order].astype(np.int16)
        dstf[c_of, slot] = (dst[order] % cp - w_of * 128).astype(np.float32)
        extra = None
        if extra_vals is not None:
            extra = np.full((N_CORES, e_slots), -1.0, np.float32)
            extra[c_of, slot] = extra_vals[order].astype(np.float32)

        a = gidx.reshape(N_CORES, e_slots // 16, 16).transpose(0, 2, 1)
        gidx_tile = np.tile(a, (1, 8, 1)).copy()  # [c, 128, J]
        dstf_tile = dstf.reshape(N_CORES, c_total, 128).transpose(0, 2, 1).copy()
        extra_tile = None
        if extra is not None:
            extra_tile = extra.reshape(N_CORES, c_total, 128).transpose(
                0, 2, 1).copy()

        sched = []
        for s in range(S):
            rows = []
            for w in range(nw):
                n = int(nchunks[s, w])
                first = int(off[s, w])
                rows.append((w, first, first + n - 1) if n else (w, -1, -2))
            sched.append(rows)
        return dict(c_total=c_total, sched=sched, gidx_tile=gidx_tile,
                    dstf_tile=dstf_tile, extra_tile=extra_tile)

    # conv1: 2 streams by src half, gather idx = src - s*half
    stream1 = (src >= half).astype(np.int64)
    p1 = bucket(stream1, src - stream1 * half)
    # conv2: single stream, gather idx = src >> 5 (32-packed p table),
    # extra = src & 31 (pair slot within the 256B row)
    p2 = bucket(np.zeros(E, np.int64), src >> 5, src & 31)

    return dict(cp=cp, nw=nw, npad=npad, half=half, p1=p1, p2=p2,
                rc_tile=rc_tile)


def build_program(plan):
    cp, nw, half, npad = plan["cp"], plan["nw"], plan["half"], plan["npad"]
    p1, p2 = plan["p1"], plan["p2"]
    c1, c2 = p1["c_total"], p2["c_total"]
    J1, J2 = c1 * 8, c2 * 8

    nc = bacc.Bacc("TRN2", target_bir_lowering=False, debug=False,
                   num_devices=N_CORES)

    def inp(name, shape, dt=F32):
        return nc.dram_tensor(name, shape, dt, kind="ExternalInput").ap()

    AGG_DT = F32R if USE_F32R else F32
    x_lo = inp("x_lo", [half, D], AGG_DT)
    x_hi = inp("x_hi", [half, D], AGG_DT)
    xt_d = inp("xt", [D + 1, cp])          # x^T with ones row
    gidx1_d = inp("gidx1", [128, J1], I16)
    dstf1_d = inp("dstf1", [128, c1])
    gidx2_d = inp("gidx2", [128, J2], I16)
    dstf2_d = inp("dstf2", [128, c2])
    kf2_d = inp("kf2", [128, c2])
    rc_d = inp("rc", [128, nw])
    iota_d = inp("iota", [128, 128])
    iota32_d = inp("iota32", [128, D])     # floor(j/2) pattern
    ident_d = inp("ident", [128, 128])
    wcb_d = inp("wcb", [D + 1, HID])       # [W1r+Wskip; b1+bskip]
    w1l_d = inp("w1l", [D, HID])
    w2lr_d = inp("w2lr", [128, 2 * 2 * OUT])  # halves of [W2l|W2r] packed
    gamma_d = inp("gamma_bc", [128, HID])
    beta_d = inp("beta_bc", [128, HID])
    b2_d = inp("b2_bc", [128, OUT])
    out_d = nc.dram_tensor("out", [cp, OUT], F32, kind="ExternalOutput").ap()

    with tile.TileContext(nc) as tc:
        with (
            tc.tile_pool(name="res", bufs=1) as res,
            tc.tile_pool(name="dram", bufs=1, space="DRAM") as dram,
        ):
            # ---- resident tiles / constants
            gidx1_sb = res.tile([128, J1], I16)
            nc.sync.dma_start(out=gidx1_sb[:], in_=gidx1_d[:])
            dstf1_sb = res.tile([128, c1], F32)
            nc.sync.dma_start(out=dstf1_sb[:], in_=dstf1_d[:])
            gidx2_sb = res.tile([128, J2], I16)
            nc.sync.dma_start(out=gidx2_sb[:], in_=gidx2_d[:])
            dstf2_sb = res.tile([128, c2], F32)
            nc.sync.dma_start(out=dstf2_sb[:], in_=dstf2_d[:])
            kf2_sb = res.tile([128, c2], F32)
            nc.sync.dma_start(out=kf2_sb[:], in_=kf2_d[:])
            rc_sb = res.tile([128, nw], F32)
            nc.sync.dma_start(out=rc_sb[:], in_=rc_d[:])
            iota_sb = res.tile([128, 128], F32)
            nc.sync.dma_start(out=iota_sb[:], in_=iota_d[:])
            iota32_sb = res.tile([128, D], F32)
            nc.sync.dma_start(out=iota32_sb[:], in_=iota32_d[:])
            ident_sb = res.tile([128, 128], F32)
            nc.sync.dma_start(out=ident_sb[:], in_=ident_d[:])
            xt_sb = res.tile([D + 1, cp], F32)
            nc.sync.dma_start(out=xt_sb[:], in_=xt_d[:])
            wcb_sb = res.tile([D + 1, HID], F32)
            nc.sync.dma_start(out=wcb_sb[:], in_=wcb_d[:])
            w1l_sb = res.tile([D, HID], F32)
            nc.sync.dma_start(out=w1l_sb[:], in_=w1l_d[:])
            w2lr_sb = res.tile([128, 2 * 2 * OUT], F32)
            nc.sync.dma_start(out=w2lr_sb[:], in_=w2lr_d[:])
            gamma_sb = res.tile([128, HID], F32)
            nc.sync.dma_start(out=gamma_sb[:], in_=gamma_d[:])
            beta_sb = res.tile([128, HID], F32)
            nc.sync.dma_start(out=beta_sb[:], in_=beta_d[:])
            b2_sb = res.tile([128, OUT], F32)
            nc.sync.dma_start(out=b2_sb[:], in_=b2_d[:])

            agg = res.tile([128, nw, D], F32)
            nc.vector.memset(agg[:], 0.0)
            pr_sb = res.tile([128, nw, 2 * OUT], F32)
            out_sb = res.tile([128, nw, OUT], F32)

            pk_local = dram.tile([cp, OUT], F32)
            pk_all = dram.tile([npad // 32, D], F32)

            # =========== conv1 aggregation + interleaved dense ===========
            sched1 = p1["sched"]
            with (
                tc.tile_pool(name="gpool", bufs=3) as gpool,
                tc.tile_pool(name="opool", bufs=8) as opool,
                tc.tile_pool(name="pwpool", bufs=3, space="PSUM") as pwp,
                tc.tile_pool(name="dwork", bufs=3) as dwork,
                tc.tile_pool(name="dsmall", bufs=4) as dsmall,
                tc.tile_pool(name="dpsum", bufs=1, space="PSUM") as dpsum,
                tc.tile_pool(name="dpsum2", bufs=1, space="PSUM") as dpsum2,
            ):
                # issue all conv1 gathers (both streams), batched
                gbufs = {}
                for s, table in ((0, x_lo), (1, x_hi)):
                    rows = [r for r in sched1[s] if r[1] >= 0]
                    if not rows:
                        continue
                    c0 = rows[0][1]
                    cend = rows[-1][2] + 1
                    for b0 in range(c0, cend, BATCH_CHUNKS):
                        b1 = min(b0 + BATCH_CHUNKS, cend)
                        g = gpool.tile([128, BATCH_CHUNKS, D], AGG_DT,
                                       tag="gbuf")
                        n_idx = (b1 - b0) * 128
                        nc.gpsimd.dma_gather(
                            out_ap=g[:, 0 : b1 - b0, :],
                            in_ap=table,
                            idxs_ap=gidx1_sb[:, b0 * 8 : b1 * 8],
                            num_idxs=n_idx,
                            num_idxs_reg=n_idx,
                            elem_size=D,
                            single_packet=False,
                        )
                        gbufs[s, b0] = g

                def accumulate(s, w, first, last, dstf_sb, gb_of):
                    """one-hot matmuls for chunks [first..last] into PSUM,
                    then add into agg[:, w]."""
                    if first > last:
                        return
                    pw = pwp.tile([128, D], F32, tag="pw")
                    for g in range(first, last + 1):
                        gb, col = gb_of(g)
                        o = opool.tile([128, 128], AGG_DT, tag="O")
                        nc.vector.tensor_scalar(
                            out=o[:], in0=iota_sb[:],
                            scalar1=dstf_sb[:, g : g + 1], scalar2=None,
                            op0=mybir.AluOpType.is_equal,
                        )
                        nc.tensor.matmul(
                            pw[:], o[:], gb[:, col, :],
                            start=(g == first), stop=(g == last),
                        )
                    nc.vector.tensor_tensor(
                        out=agg[:, w, :], in0=agg[:, w, :], in1=pw[:],
                        op=mybir.AluOpType.add,
                    )

                def gb_of1(s):
                    rows = [r for r in sched1[s] if r[1] >= 0]
                    c0 = rows[0][1]

                    def f(g):
                        b0 = c0 + ((g - c0) // BATCH_CHUNKS) * BATCH_CHUNKS
                        return gbufs[s, b0], g - b0
                    return f

                # stream 0: plain accumulation
                if any(r[1] >= 0 for r in sched1[0]):
                    f0 = gb_of1(0)
                    for w, first, last in sched1[0]:
                        accumulate(0, w, first, last, dstf1_sb, f0)

                # stream 1: accumulate, then dense phase for that window
                f1 = gb_of1(1) if any(r[1] >= 0 for r in sched1[1]) else None
                for w, first, last in sched1[1]:
                    if f1 is not None:
                        accumulate(1, w, first, last, dstf1_sb, f1)

                    # ---------- dense phase for window w ----------
                    mean_n = dwork.tile([128, D], F32, tag="mean")
                    nc.vector.tensor_scalar(
                        out=mean_n[:], in0=agg[:, w, :],
                        scalar1=rc_sb[:, w : w + 1], scalar2=None,
                        op0=mybir.AluOpType.mult,
                    )
                    tp2 = dpsum.tile([D, 128], F32, tag="tp")
                    nc.tensor.transpose(tp2[:], mean_n[:], ident_sb[:])
                    meant = dwork.tile([D, 128], F32, tag="meant")
                    nc.scalar.activation(
                        meant[:], tp2[:], mybir.ActivationFunctionType.Copy)

                    # x1 = x@Wc + bc + mean@W1l   [128, HID]
                    x1p = dpsum2.tile([128, HID], F32, tag="x1")
                    nc.tensor.matmul(
                        x1p[:], xt_sb[:, 128 * w : 128 * (w + 1)], wcb_sb[:],
                        start=True, stop=False)
                    nc.tensor.matmul(x1p[:], meant[:], w1l_sb[:],
                                     start=False, stop=True)

                    # LayerNorm + ELU
                    mu = dsmall.tile([128, 1], F32, tag="mu")
                    nc.vector.reduce_sum(out=mu[:], in_=x1p[:],
                                         axis=mybir.AxisListType.X)
                    nc.vector.tensor_scalar(
                        out=mu[:], in0=mu[:], scalar1=1.0 / HID,
                        scalar2=None, op0=mybir.AluOpType.mult)
                    xc = dwork.tile([128, HID], F32, tag="xc")
                    nc.vector.tensor_scalar(
                        out=xc[:], in0=x1p[:], scalar1=mu[:], scalar2=None,
                        op0=mybir.AluOpType.subtract)
                    sq = dwork.tile([128, HID], F32, tag="sq")
                    var = dsmall.tile([128, 1], F32, tag="var")
                    nc.scalar.activation(
                        sq[:], xc[:], mybir.ActivationFunctionType.Square,
                        accum_out=var[:])
                    rstd = dsmall.tile([128, 1], F32, tag="rstd")
                    nc.vector.tensor_scalar(
                        out=rstd[:], in0=var[:], scalar1=1.0 / HID,
                        scalar2=LN_EPS, op0=mybir.AluOpType.mult,
                        op1=mybir.AluOpType.add)
                    nc.scalar.activation(
                        rstd[:], rstd[:], mybir.ActivationFunctionType.Sqrt)
                    nc.vector.reciprocal(rstd[:], rstd[:])
                    y = dwork.tile([128, HID], F32, tag="y")
                    nc.vector.scalar_tensor_tensor(
                        out=y[:], in0=xc[:], scalar=rstd[:], in1=gamma_sb[:],
                        op0=mybir.AluOpType.mult, op1=mybir.AluOpType.mult)
                    nc.vector.tensor_tensor(
                        out=y[:], in0=y[:], in1=beta_sb[:],
                        op=mybir.AluOpType.add)
                    # ELU: h = (max(y,0)-1) + exp(min(y,0))
                    m0 = dwork.tile([128, HID], F32, tag="m0")
                    nc.vector.tensor_scalar(
                        out=m0[:], in0=y[:], scalar1=0.0, scalar2=None,
                        op0=mybir.AluOpType.min)
                    ex = dwork.tile([128, HID], F32, tag="ex")
                    nc.scalar.activation(
                        ex[:], m0[:], mybir.ActivationFunctionType.Exp)
                    rm1 = dwork.tile([128, HID], F32, tag="rm1")
                    nc.vector.tensor_scalar(
                        out=rm1[:], in0=y[:], scalar1=0.0, scalar2=-1.0,
                        op0=mybir.AluOpType.max, op1=mybir.AluOpType.add)
                    h = dwork.tile([128, HID], F32, tag="h")
                    nc.vector.tensor_tensor(
                        out=h[:], in0=rm1[:], in1=ex[:],
                        op=mybir.AluOpType.add)

                    # p | r = h @ [W2l | W2r]
                    prp = dpsum2.tile([128, 2 * OUT], F32, tag="pr")
                    for hh in range(2):
                        tph = dpsum.tile([128, 128], F32, tag="tph")
                        nc.tensor.transpose(
                            tph[:], h[:, 128 * hh : 128 * (hh + 1)],
                            ident_sb[:])
                        hts = dwork.tile([128, 128], F32, tag="hts")
                        nc.scalar.activation(
                            hts[:], tph[:], mybir.ActivationFunctionType.Copy)
                        nc.tensor.matmul(
                            prp[:], hts[:],
                            w2lr_sb[:, 4 * hh : 4 * (hh + 1)],
                            start=(hh == 0), stop=(hh == 1))
                    nc.scalar.activation(
                        pr_sb[:, w, :], prp[:],
                        mybir.ActivationFunctionType.Copy)

                # p (node-major pairs) -> DRAM for the allgather
                nc.sync.dma_start(
                    out=pk_local.rearrange("(w p) c -> p w c", p=128),
                    in_=pr_sb[:, :, 0:OUT])

            # ================= p all-gather (50KB per core) =================
            nc.gpsimd.collective_compute(
                "AllGather",
                mybir.AluOpType.bypass,
                replica_groups=[list(range(N_CORES))],
                ins=[pk_local.opt()],
                outs=[pk_all.opt()],
            )

            # ================= conv2 aggregation =================
            sched2 = p2["sched"][0]
            with (
                tc.tile_pool(name="g2pool", bufs=3) as g2pool,
                tc.tile_pool(name="o2pool", bufs=8) as o2pool,
                tc.tile_pool(name="m2pool", bufs=8) as m2pool,
                tc.tile_pool(name="pw2pool", bufs=3, space="PSUM") as pw2p,
                tc.tile_pool(name="fwork", bufs=4) as fwork,
            ):
                rows = [r for r in sched2 if r[1] >= 0]
                c0 = rows[0][1]
                cend = rows[-1][2] + 1
                g2bufs = {}
                for b0 in range(c0, cend, BATCH_CHUNKS):
                    b1 = min(b0 + BATCH_CHUNKS, cend)
                    g = g2pool.tile([128, BATCH_CHUNKS, D], AGG_DT,
                                    tag="g2buf")
                    n_idx = (b1 - b0) * 128
                    nc.gpsimd.dma_gather(
                        out_ap=g[:, 0 : b1 - b0, :],
                        in_ap=pk_all[:].bitcast(AGG_DT) if USE_F32R
                        else pk_all,
                        idxs_ap=gidx2_sb[:, b0 * 8 : b1 * 8],
                        num_idxs=n_idx,
                        num_idxs_reg=n_idx,
                        elem_size=D,
                        single_packet=False,
                    )
                    g2bufs[b0] = g

                for w, first, last in sched2:
                    if first > last:
                        nc.vector.tensor_tensor(
                            out=out_sb[:, w, :],
                            in0=pr_sb[:, w, OUT : 2 * OUT], in1=b2_sb[:],
                            op=mybir.AluOpType.add)
                        continue
                    pw2 = pw2p.tile([128, D], F32, tag="pw2")
                    for g in range(first, last + 1):
                        b0 = c0 + ((g - c0) // BATCH_CHUNKS) * BATCH_CHUNKS
                        gb = g2bufs[b0]
                        # masked row: (iota32 == k) * gathered
                        mt = m2pool.tile([128, D], AGG_DT, tag="mt")
                        nc.vector.scalar_tensor_tensor(
                            out=mt[:], in0=iota32_sb[:],
                            scalar=kf2_sb[:, g : g + 1],
                            in1=gb[:, g - b0, :],
                            op0=mybir.AluOpType.is_equal,
                            op1=mybir.AluOpType.mult)
                        o = o2pool.tile([128, 128], AGG_DT, tag="O2")
                        nc.vector.tensor_scalar(
                            out=o[:], in0=iota_sb[:],
                            scalar1=dstf2_sb[:, g : g + 1], scalar2=None,
                            op0=mybir.AluOpType.is_equal,
                        )
                        nc.tensor.matmul(
                            pw2[:], o[:], mt[:],
                            start=(g == first), stop=(g == last),
                        )
                    # tree-reduce the 32 packed slots: [128,64] -> [128,2]
                    s64 = fwork.tile([128, D], F32, tag="s64")
                    nc.scalar.activation(
                        s64[:], pw2[:], mybir.ActivationFunctionType.Copy)
                    t32 = fwork.tile([128, 32], F32, tag="t32")
                    nc.vector.tensor_tensor(
                        out=t32[:], in0=s64[:, 0:32], in1=s64[:, 32:64],
                        op=mybir.AluOpType.add)
                    t16 = fwork.tile([128, 16], F32, tag="t16")
                    nc.vector.tensor_tensor(
                        out=t16[:], in0=t32[:, 0:16], in1=t32[:, 16:32],
                        op=mybir.AluOpType.add)
                    t8 = fwork.tile([128, 8], F32, tag="t8")
                    nc.vector.tensor_tensor(
                        out=t8[:], in0=t16[:, 0:8], in1=t16[:, 8:16],
                        op=mybir.AluOpType.add)
                    t4 = fwork.tile([128, 4], F32, tag="t4")
                    nc.vector.tensor_tensor(
                        out=t4[:], in0=t8[:, 0:4], in1=t8[:, 4:8],
                        op=mybir.AluOpType.add)
                    t2 = fwork.tile([128, 2], F32, tag="t2")
                    nc.vector.tensor_tensor(
                        out=t2[:], in0=t4[:, 0:2], in1=t4[:, 2:4],
                        op=mybir.AluOpType.add)
                    # out = t2 * rc + r + b2
                    t = fwork.tile([128, OUT], F32, tag="fo")
                    nc.vector.tensor_scalar(
                        out=t[:], in0=t2[:], scalar1=rc_sb[:, w : w + 1],
                        scalar2=None, op0=mybir.AluOpType.mult)
                    nc.vector.tensor_tensor(
                        out=t[:], in0=t[:], in1=pr_sb[:, w, OUT : 2 * OUT],
                        op=mybir.AluOpType.add)
                    nc.vector.tensor_tensor(
                        out=out_sb[:, w, :], in0=t[:], in1=b2_sb[:],
                        op=mybir.AluOpType.add)

            nc.sync.dma_start(
                out=out_d.rearrange("(w p) c -> p w c", p=128),
                in_=out_sb[:])

    nc.compile()
    return nc


def make_inputs(plan, x, W1l, W1r, b1, Wskip, bskip, gamma, beta, W2l, W2r,
                b2, n_nodes):
    cp, half, npad, nw = plan["cp"], plan["half"], plan["npad"], plan["nw"]
    xp = np.zeros((npad, D), np.float32)
    xp[:n_nodes] = np.asarray(x, np.float32)
    wc = np.asarray(W1r, np.float32) + np.asarray(Wskip, np.float32)
    bc = np.asarray(b1, np.float32) + np.asarray(bskip, np.float32)
    wcb = np.concatenate([wc, bc[None, :]], axis=0)
    w2lr_full = np.concatenate(
        [np.asarray(W2l, np.float32), np.asarray(W2r, np.float32)], axis=1
    )  # [HID, 4]
    w2lr = (
        w2lr_full.reshape(2, 128, 2 * OUT).transpose(1, 0, 2)
        .reshape(128, 2 * 2 * OUT).copy()
    )
    iota = np.tile(np.arange(128, dtype=np.float32)[None, :], (128, 1))
    iota32 = np.tile(
        (np.arange(D, dtype=np.float32) // 2)[None, :], (128, 1))
    ident = np.eye(128, dtype=np.float32)
    gamma_bc = np.tile(np.asarray(gamma, np.float32)[None, :], (128, 1))
    beta_bc = np.tile(np.asarray(beta, np.float32)[None, :], (128, 1))
    b2_bc = np.tile(np.asarray(b2, np.float32)[None, :], (128, 1))

    common = dict(
        x_lo=xp[:half].copy(), x_hi=xp[half:].copy(),
        iota=iota, iota32=iota32, ident=ident,
        wcb=wcb, w1l=np.asarray(W1l, np.float32), w2lr=w2lr,
        gamma_bc=gamma_bc, beta_bc=beta_bc, b2_bc=b2_bc,
    )
    in_maps = []
    for c in range(N_CORES):
        m = dict(common)
        xc_loc = xp[cp * c : cp * (c + 1)]
        xt = np.empty((D + 1, cp), np.float32)
        xt[0:D] = xc_loc.T
        xt[D] = 1.0
        m["xt"] = xt
        m["gidx1"] = plan["p1"]["gidx_tile"][c]
        m["dstf1"] = plan["p1"]["dstf_tile"][c]
        m["gidx2"] = plan["p2"]["gidx_tile"][c]
        m["dstf2"] = plan["p2"]["dstf_tile"][c]
        m["kf2"] = plan["p2"]["extra_tile"][c]
        m["rc"] = plan["rc_tile"][c]
        in_maps.append(m)
    return in_maps


_CACHE = {}


def _get_compiled(edge_index, n_nodes):
    key = (edge_index.tobytes()[:512], edge_index.shape, n_nodes)
    if key not in _CACHE:
        plan = make_plan(edge_index, n_nodes)
        nc = build_program(plan)
        _CACHE[key] = (plan, nc)
    return _CACHE[key]


def run(inputs, trace=False):
    x = np.asarray(inputs["x"], np.float32)
    edge_index = np.asarray(inputs["edge_index"], np.int32)
    n_nodes = x.shape[0]
    plan, nc = _get_compiled(edge_index, n_nodes)
    in_maps = make_inputs(
        plan, x, inputs["W1l"], inputs["W1r"], inputs["b1"], inputs["Wskip"],
        inputs["bskip"], inputs["gamma"], inputs["beta"], inputs["W2l"],
        inputs["W2r"], inputs["b2"], n_nodes)
    res = run_bass_kernel_spmd(
        nc, in_maps, list(range(N_CORES)), trace=trace)
    cp = plan["cp"]
    out = np.empty((n_nodes, OUT), np.float32)
    for c in range(N_CORES):
        lo = cp * c
        hi = min(cp * (c + 1), n_nodes)
        out[lo:hi] = res.results[c]["out"][0 : hi - lo]
    return out, res


def kernel(**inputs) -> np.ndarray:
    out, _ = run(inputs)
    return out
